# revision 18
# baseline (speedup 1.0000x reference)
"""Bilateral filter (d=5, sigmaColor=0.1, sigmaSpace=1) Trainium2 Bass kernel, v4.

Full inputs in, full outputs out. Data-parallel over 8 NeuronCores: 2 images
per core. Per-core layout: partitions = (img, row-block-of-8); each partition
stores, in fp16, a planar [ch][12 rows][Wp+4 cols] center tile whose 2-row /
2-col halos make every 5x5 window offset a pure free-dim AP shift. Processed
in column passes of Wp=256.

v4 reformulation: offsets processed in symmetric PAIRS {+o, -o} using
    w_o(q)   = exp(-50*||I(q+o)-I(q)||^2 - s_o/2)   (w_{-o}(p) = w_o(p-o))
    d_o(q)   = I(q+o) - I(q),   v_o = w_o * d_o
    out(p)   = I(p) + [sum_pairs (v_o(p) - v_o(p-o))] / den(p)
    den(p)   = 1 + sum_pairs (w_o(p) + w_o(p-o))
Each pair's w/v are computed once on a slightly extended domain (dy extra
rows, |dx| extra cols) and read twice (at p and p-o), halving the exp/square
work and skipping the explicit num accumulation of w*nbr (padded zeros fall
out exactly). All DVE tensor_tensor ops run in fp16 2x mode; ACT does
squares/exp; GPSIMD only accumulates den.
"""

import os
import sys

import numpy as np

for _p in ("/opt/trn_rl_repo",):
    if os.path.isdir(_p) and _p not in sys.path:
        sys.path.append(_p)

import concourse.bacc as bacc
import concourse.bass as bass
import concourse.tile as tile
from concourse import mybir
from concourse.ap import AP
from concourse.bass_utils import run_bass_kernel_spmd

F16 = mybir.dt.float16
F32 = mybir.dt.float32
ALU = mybir.AluOpType
ACTF = mybir.ActivationFunctionType

N_CORES = 8
R = 2  # window radius

# 12 pair representatives (dy>0, or dy==0 and dx>0), largest s first so the
# biggest weights are accumulated last.
PAIRS = sorted(
    [(dy, dx) for dy in range(0, R + 1) for dx in range(-R, R + 1)
     if dy > 0 or dx > 0],
    key=lambda o: -(o[0] * o[0] + o[1] * o[1]),
)

USE_GP_DEN = False  # GPSIMD concurrency halves DVE throughput — keep it idle


class Cfg:
    def __init__(self, B=2, H=512, W=512, Wp=256):
        self.B, self.H, self.W, self.Wp = B, H, W, Wp
        self.C = 3
        self.RBR = 8                      # core rows per partition
        self.RBN = H // self.RBR          # row blocks per image
        self.P = B * self.RBN             # partitions
        self.RH = self.RBR + 2 * R        # stored rows (12)
        self.WS = Wp + 2 * R              # stored cols per pass
        self.NPASS = W // Wp
        assert H % self.RBR == 0 and W % Wp == 0 and self.P <= 128
        assert self.WS % 2 == 0


FULL = Cfg()


def build(cfg: Cfg, enable_asserts=False, repeat=1):
    B, H, W, Wp, C = cfg.B, cfg.H, cfg.W, cfg.Wp, cfg.C
    P, RBN, RBR, RH, WS = cfg.P, cfg.RBN, cfg.RBR, cfg.RH, cfg.WS
    WC = W * C          # f32 elems per image row in DRAM
    HWC = H * WC

    nc = bacc.Bacc(
        "TRN2",
        target_bir_lowering=False,
        debug=False,
        enable_asserts=enable_asserts,
        num_devices=N_CORES,
    )
    for bv in sorted({-0.5 * float(dy * dy + dx * dx) for dy, dx in PAIRS}):
        t = nc.alloc_sbuf_tensor(f"const-bias-{bv}", [128, 1], F32)
        nc.gpsimd.memset(t.ap(), bv)
        nc.const_aps.aps[(F32, bv)] = t.ap()
    nc.all_engine_barrier()

    x_h = nc.dram_tensor("x", [B, H, W, C], F32, kind="ExternalInput")
    y_h = nc.dram_tensor("out", [B, H, W, C], F32, kind="ExternalOutput")
    x_flat = x_h.ap().rearrange("b h w c -> (b h w c)")
    y_flat = y_h.ap().rearrange("b h w c -> (b h w c)")

    def dram_ap(flat, offset, dims):
        return AP(flat.tensor, offset, [list(d) for d in dims])

    with tile.TileContext(nc) as tc:
        with (
            tc.tile_pool(name="state", bufs=1) as state_pool,
            tc.tile_pool(name="dpool", bufs=3) as dpool,
            tc.tile_pool(name="sqpool", bufs=2) as sqpool,
            tc.tile_pool(name="wpool", bufs=3) as wpool,
            tc.tile_pool(name="v3pool", bufs=2) as v3pool,
            tc.tile_pool(name="io", bufs=2) as io_pool,
            tc.tile_pool(name="tail", bufs=1) as tail_pool,
        ):
            zt = state_pool.tile([P, 2 * WC], F16, name="z", tag="zt")
            nc.vector.memset(zt[:, :], 0.0)

            def load_pass(ps, ps0):
                """Emit DMA loads for pass ps0; returns (Ct, [deint emitters]).

                The main row-block DMAs are split into 3-row halves so each
                deinterleave copy (emitted later, spread over the ACT queue)
                only waits for its own half (sub-tile deps)."""
                c_lo = ps0 * Wp
                v_lo = max(0, c_lo - R)
                v_hi = min(W, c_lo + Wp + R)
                nv = v_hi - v_lo
                s_lo = v_lo - c_lo + R
                Ct = state_pool.tile([P, C, RH, WS], F16, name=f"C_{ps}",
                                     tag=f"C{ps % 2}")
                if s_lo > 0:
                    nc.vector.memset(Ct[:, :, :, 0:s_lo], 0.0)
                if s_lo + nv < WS:
                    nc.vector.memset(Ct[:, :, :, s_lo + nv: WS], 0.0)
                deints = []
                nvc = nv * C        # loaded elems per row (col-sliced)
                for ck in range(2):  # stored rows [6*ck, 6*ck+6)
                    r0 = 6 * ck
                    St = io_pool.tile([P, 6 * nvc], F16, name=f"S_{ps}_{ck}", tag="S")
                    rb_a = 1 if ck == 0 else 0
                    rb_b = RBN if ck == 0 else RBN - 1
                    for img in range(B):
                        pb = img * RBN
                        row0 = 8 * rb_a - 2 + r0
                        for hf in range(2):
                            nc.gpsimd.dma_start(
                                out=St[pb + rb_a: pb + rb_b,
                                       hf * 3 * nvc: (hf + 1) * 3 * nvc],
                                in_=dram_ap(
                                    x_flat,
                                    img * HWC + (row0 + 3 * hf) * WC + v_lo * C,
                                    [(8 * WC, rb_b - rb_a), (WC, 3), (1, nvc)],
                                ),
                            )
                        if ck == 0:
                            nc.sync.dma_start(
                                out=St[pb: pb + 1, 0: 2 * nvc],
                                in_=zt[pb: pb + 1, 0: 2 * nvc],
                            )
                            nc.gpsimd.dma_start(
                                out=St[pb: pb + 1, 2 * nvc: 6 * nvc],
                                in_=dram_ap(
                                    x_flat, img * HWC + v_lo * C,
                                    [(8 * WC, 1), (WC, 4), (1, nvc)],
                                ),
                            )
                        else:
                            pe = pb + RBN - 1
                            nc.sync.dma_start(
                                out=St[pe: pe + 1, 4 * nvc: 6 * nvc],
                                in_=zt[pe: pe + 1, 0: 2 * nvc],
                            )
                            nc.gpsimd.dma_start(
                                out=St[pe: pe + 1, 0: 4 * nvc],
                                in_=dram_ap(
                                    x_flat,
                                    img * HWC + (H - 4) * WC + v_lo * C,
                                    [(8 * WC, 1), (WC, 4), (1, nvc)],
                                ),
                            )
                    # deinterleave halves: C[ch, r0+3h+r, s_lo+w] = S[3h+r, w, ch]
                    s_v = St[:, :].rearrange("p (r w c) -> p c r w", r=6, w=nv, c=C)

                    def mk(ck=ck, r0=r0, s_v=s_v):
                        def emit(hf):
                            nc.scalar.copy(
                                Ct[:, :, r0 + 3 * hf: r0 + 3 * hf + 3,
                                   s_lo: s_lo + nv],
                                s_v[:, :, 3 * hf: 3 * hf + 3, :],
                            )
                        return emit
                    deints.append(mk())
                return Ct, deints

            Ct_next, dn = load_pass(0, 0)
            for _d in dn:
                _d(0)
                _d(1)
            for rep in range(repeat):
              for ps0 in range(cfg.NPASS):
                ps = rep * cfg.NPASS + ps0
                c_lo = ps0 * Wp         # first core image col of this pass

                Ct = Ct_next
                num = state_pool.tile([P, C, RBR, Wp], F16, name=f"num_{ps}",
                                      tag="num")
                den = state_pool.tile([P, RBR, Wp], F16, name=f"den_{ps}",
                                      tag="den")
                denB = state_pool.tile([P, RBR, Wp], F16, name=f"denB_{ps}",
                                       tag="denB")
                nbP = state_pool.tile([P, RBR, Wp], F16, name=f"nbP_{ps}",
                                      tag="nbP")

                ctr = Ct[:, :, R: R + RBR, R: R + Wp]
                den2 = den[:, :, :]

                # --- 12 symmetric pairs, software-pipelined by one stage ---
                # stage A(k): d = shifted diff            (DVE)
                # stage B(k): sq, cd, w = exp             (ACT+DVE+ACT)
                # stage C(k): v = w*d ; num +-= v ; den += w  (DVE+GP)
                ctx = {}

                MR, MC = RBR + R, Wp + R   # max ext tile dims (10, 258)

                def stage_A(k):
                    dy, dx = PAIRS[k]
                    nr = RBR + dy
                    ncl = Wp + abs(dx)
                    rlo = R - dy
                    clo = R - max(dx, 0)
                    D = dpool.tile([P, C, MR, MC], F16, name=f"D_{ps}_{k}", tag="D")
                    q0 = Ct[:, :, rlo: rlo + nr, clo: clo + ncl]
                    q1 = Ct[:, :, rlo + dy: rlo + dy + nr,
                            clo + dx: clo + dx + ncl]
                    nc.vector.tensor_sub(D[:, :, 0:nr, 0:ncl], q1, q0)
                    ctx[k] = (D, nr, ncl, rlo, clo)

                def stage_B(k):
                    dy, dx = PAIRS[k]
                    D, nr, ncl, rlo, clo = ctx[k]
                    SA = sqpool.tile([P, MR, MC], F16, name=f"SA_{ps}_{k}", tag="SA")
                    SB = sqpool.tile([P, MR, MC], F16, name=f"SB_{ps}_{k}", tag="SB")
                    sa, sb = SA[:, 0:nr, 0:ncl], SB[:, 0:nr, 0:ncl]
                    nc.scalar.activation(sa, D[:, 0, 0:nr, 0:ncl], ACTF.Square)
                    nc.scalar.activation(sb, D[:, 1, 0:nr, 0:ncl], ACTF.Square)
                    nc.vector.tensor_add(sa, sa, sb)
                    nc.scalar.activation(sb, D[:, 2, 0:nr, 0:ncl], ACTF.Square)
                    nc.vector.tensor_add(sa, sa, sb)
                    Wt = wpool.tile([P, MR, MC], F16, name=f"W_{ps}_{k}", tag="W")
                    nc.scalar.activation(
                        Wt[:, 0:nr, 0:ncl], sa, ACTF.Exp,
                        bias=-0.5 * float(dy * dy + dx * dx), scale=-50.0,
                    )
                    ctx[k] = (D, Wt, nr, ncl, rlo, clo)

                def stage_C(k):
                    dy, dx = PAIRS[k]
                    D, Wt, nr, ncl, rlo, clo = ctx.pop(k)
                    wv = Wt[:, 0:nr, 0:ncl]
                    wb2 = wv.unsqueeze(1).broadcast_to((P, 2, nr, ncl))
                    d2 = D[:, 0:2, 0:nr, 0:ncl]
                    nc.vector.tensor_mul(d2, wb2, d2)
                    # channel-2 v goes to its own tile so the nbP DMA chain
                    # can read it without pinning the D pool
                    V3 = v3pool.tile([P, MR, MC], F16, name=f"V3_{ps}_{k}",
                                     tag="V3")
                    nc.vector.tensor_mul(V3[:, 0:nr, 0:ncl], wv,
                                         D[:, 2, 0:nr, 0:ncl])
                    # local coords of core (p) and shifted (p-o) windows
                    rc, cc = R - rlo, R - clo
                    rs, cs = rc - dy, cc - dx
                    v2_p = D[:, 0:2, rc: rc + RBR, cc: cc + Wp]
                    v2_m = D[:, 0:2, rs: rs + RBR, cs: cs + Wp]
                    v3_p = V3[:, rc: rc + RBR, cc: cc + Wp]
                    v3_m = V3[:, rs: rs + RBR, cs: cs + Wp]
                    w_p = Wt[:, rc: rc + RBR, cc: cc + Wp]
                    w_m = Wt[:, rs: rs + RBR, cs: cs + Wp]
                    n2 = num[:, 0:2, :, :]
                    n3 = num[:, 2, :, :]
                    if k == 0:
                        nc.vector.tensor_sub(n2, v2_p, v2_m)
                        nc.vector.tensor_sub(n3, v3_p, v3_m)
                        nc.vector.tensor_add(den2, w_p, w_m)
                    elif k == 1:
                        nc.vector.tensor_add(n2, n2, v2_p)
                        nc.vector.tensor_sub(n2, n2, v2_m)
                        nc.vector.tensor_add(n3, n3, v3_p)
                        nc.vector.tensor_sub(n3, n3, v3_m)
                        nc.vector.tensor_add(denB[:, :, :], w_p, w_m)
                    elif k < NP - 2:
                        nc.vector.tensor_add(n2, n2, v2_p)
                        nc.vector.tensor_sub(n2, n2, v2_m)
                        nc.vector.tensor_sub(n3, n3, v3_m)
                        # DMA compute engines accumulate den (both sides) and
                        # the channel-2 +v side — off both DVE and GPSIMD cores
                        if k == 2:
                            nc.gpsimd.dma_start(out=nbP[:, :, :], in_=v3_p)
                        else:
                            nc.gpsimd.dma_start(out=nbP[:, :, :], in_=v3_p,
                                                accum_op=ALU.add)
                        nc.gpsimd.dma_start(out=den2, in_=w_p, accum_op=ALU.add)
                        nc.gpsimd.dma_start(out=denB[:, :, :], in_=w_m,
                                            accum_op=ALU.add)
                    else:
                        # last pairs: accumulate on DVE so the tail doesn't
                        # wait on the DMA chains
                        nc.vector.tensor_add(n2, n2, v2_p)
                        nc.vector.tensor_sub(n2, n2, v2_m)
                        nc.vector.tensor_add(n3, n3, v3_p)
                        nc.vector.tensor_sub(n3, n3, v3_m)
                        nc.vector.tensor_add(den2, den2, w_p)
                        nc.vector.tensor_add(denB[:, :, :], denB[:, :, :], w_m)

                NP = len(PAIRS)
                is_last = (rep == repeat - 1) and (ps0 == cfg.NPASS - 1)
                stage_A(0)
                stage_B(0)
                stage_A(1)
                for k in range(NP):
                    if k + 2 < NP:
                        stage_A(k + 2)
                    stage_C(k)
                    if k + 1 < NP:
                        stage_B(k + 1)
                    if not is_last:
                        # prefetch next pass: DMAs at k==5, then the four
                        # deinterleave halves spread over the ACT queue
                        if k == 5:
                            nps = ps + 1
                            Ct_next, dn = load_pass(nps, nps % cfg.NPASS)
                        elif 6 <= k <= 9:
                            dn[(k - 6) // 2]((k - 6) % 2)

                # --- tail: out = ctr + num * (1 / (den + 1)) ---
                # merge the DMA-accumulated channel-2 chain, then process in
                # row chunks so recip/normalize/reinterleave/store pipeline
                nc.vector.tensor_add(num[:, 2, :, :], num[:, 2, :, :],
                                     nbP[:, :, :])
                HR = RBR // 4
                den1 = tail_pool.tile([P, RBR * Wp], F16, name=f"d1_{ps}", tag="d1", bufs=1)
                denf = tail_pool.tile([P, RBR * Wp], F32, name=f"df_{ps}", tag="df", bufs=1)
                rden16 = tail_pool.tile([P, RBR * Wp], F16, name=f"r16_{ps}", tag="d1", bufs=1)
                Oi = io_pool.tile([P, RBR * Wp * C], F16, name=f"Oi_{ps}", tag="S")
                r16_3 = rden16.rearrange("p (r w) -> p r w", r=RBR, w=Wp)
                den1_3 = den1.rearrange("p (r w) -> p r w", r=RBR, w=Wp)
                o_v = Oi[:, :].rearrange("p (r w c) -> p r w c", r=RBR, w=Wp, c=C)
                n_v = num.rearrange("p c r w -> p r w c")
                for h in range(4):
                    rs = slice(h * HR, (h + 1) * HR)
                    fs = slice(h * HR * Wp, (h + 1) * HR * Wp)
                    nc.vector.tensor_add(den[:, rs, :], den[:, rs, :],
                                         denB[:, rs, :])
                    nc.scalar.activation(den1_3[:, rs, :], den[:, rs, :],
                                         ACTF.Copy, bias=1.0)
                    nc.scalar.copy(denf[:, fs], den1[:, fs])
                    nc.vector.reciprocal_approx_fast(denf[:, fs], denf[:, fs])
                    nc.scalar.copy(rden16[:, fs], denf[:, fs])
                    rbb = r16_3[:, rs, :].unsqueeze(1).broadcast_to(
                        (P, C, HR, Wp))
                    nc.vector.tensor_mul(num[:, :, rs, :], rbb,
                                         num[:, :, rs, :])
                    nc.vector.tensor_add(num[:, :, rs, :], num[:, :, rs, :],
                                         ctr[:, :, rs, :])
                    # reinterleave (dst innermost unit-stride) + store
                    nc.scalar.copy(o_v[:, rs, :, :], n_v[:, rs, :, :])
                    for img in range(B):
                        pb = img * RBN
                        nc.gpsimd.dma_start(
                            out=dram_ap(
                                y_flat,
                                img * HWC + (h * HR) * WC + c_lo * C,
                                [(8 * WC, RBN), (WC, HR), (1, Wp * C)],
                            ),
                            in_=Oi[pb: pb + RBN,
                                   h * HR * Wp * C: (h + 1) * HR * Wp * C
                                   ].rearrange(
                                "p (r w) -> p r w", r=HR, w=Wp * C
                            ),
                        )

    nc.compile()
    return nc


def make_timed_fn(nc, in_maps, n_cores=N_CORES):
    """Jitted sharded executor over device-resident inputs for wall-clock
    benchmarking."""
    import jax
    from jax.sharding import Mesh, PartitionSpec
    from jax.experimental.shard_map import shard_map
    import concourse.bass2jax as b2j
    from concourse import mybir as _mb

    b2j.install_neuronx_cc_hook()
    partition_name = nc.partition_id_tensor.name if nc.partition_id_tensor else None
    in_names, out_names, out_avals = [], [], []
    for alloc in nc.m.functions[0].allocations:
        if not isinstance(alloc, _mb.MemoryLocationSet):
            continue
        name = alloc.memorylocations[0].name
        if alloc.kind == "ExternalInput":
            if name != partition_name:
                in_names.append(name)
        elif alloc.kind == "ExternalOutput":
            out_names.append(name)
            out_avals.append(
                jax.core.ShapedArray(tuple(alloc.tensor_shape), _mb.dt.np(alloc.dtype))
            )
    n_params = len(in_names)
    zero_outs = [np.zeros(a.shape, a.dtype) for a in out_avals]
    all_in_names = list(in_names) + list(out_names)
    if partition_name is not None:
        all_in_names.append(partition_name)
    if nc.dbg_addr is not None:
        in_maps = [
            {**m, nc.dbg_addr.name: np.zeros((1, 2), np.uint32)} for m in in_maps
        ]
        if nc.dbg_addr.name not in in_names:
            in_names.append(nc.dbg_addr.name)
            all_in_names.insert(len(in_names) - 1, nc.dbg_addr.name)
            n_params += 1

    def _body(*args):
        operands = list(args)
        if partition_name is not None:
            operands.append(b2j.partition_id_tensor())
        return tuple(
            b2j._bass_exec_p.bind(
                *operands,
                out_avals=tuple(out_avals),
                in_names=tuple(all_in_names),
                out_names=tuple(out_names),
                lowering_input_output_aliases=(),
                sim_require_finite=True,
                sim_require_nnan=True,
                nc=nc,
            )
        )

    devices = jax.devices()[:n_cores]
    mesh = Mesh(np.asarray(devices), ("core",))
    n_outs = len(out_names)
    sharded = jax.jit(
        shard_map(
            _body,
            mesh=mesh,
            in_specs=(PartitionSpec("core"),) * (n_params + n_outs),
            out_specs=(PartitionSpec("core"),) * n_outs,
            check_rep=False,
        ),
        keep_unused=True,
    )
    concat_in = [
        np.concatenate([np.asarray(m[name]) for m in in_maps], axis=0)
        for name in in_names
    ]
    concat_zero = [
        np.zeros((n_cores * z.shape[0], *z.shape[1:]), z.dtype) for z in zero_outs
    ]
    sharding = jax.sharding.NamedSharding(mesh, PartitionSpec("core"))
    dev_args = [jax.device_put(a, sharding) for a in concat_in + concat_zero]

    def run():
        outs = sharded(*dev_args)
        jax.block_until_ready(outs)
        return outs

    return run


def bench(x=None, iters=6, repeats=(1, 5)):
    import time as _t

    if x is None:
        rng = np.random.default_rng(0)
        x = rng.random((16, 512, 512, 3), dtype=np.float32)
    x = np.ascontiguousarray(np.asarray(x), np.float32)
    bpc = x.shape[0] // N_CORES
    in_maps = [{"x": x[i * bpc: (i + 1) * bpc]} for i in range(N_CORES)]
    times = {}
    for rep in repeats:
        nc = build(FULL, repeat=rep)
        fn = make_timed_fn(nc, in_maps)
        fn()
        fn()
        ts = []
        for _ in range(iters):
            t0 = _t.perf_counter()
            fn()
            ts.append(_t.perf_counter() - t0)
        times[rep] = min(ts)
        print(f"repeat={rep}: min wall {times[rep]*1e6:.0f} us over {iters} iters")
    r0, r1 = repeats
    hw_ns = (times[r1] - times[r0]) / (r1 - r0) * 1e9
    print(f"HW exec time: {hw_ns:.0f} ns")
    return hw_ns


_NC_CACHE = {}


def _get_nc():
    if "full" not in _NC_CACHE:
        _NC_CACHE["full"] = build(FULL)
    return _NC_CACHE["full"]


def kernel(x, trace=False, **_ignored):
    x = np.ascontiguousarray(np.asarray(x), dtype=np.float32)
    B = x.shape[0]
    bpc = B // N_CORES
    nc = _get_nc()
    in_maps = [{"x": x[i * bpc: (i + 1) * bpc]} for i in range(N_CORES)]
    res = run_bass_kernel_spmd(nc, in_maps, list(range(N_CORES)), trace=trace)
    out = np.concatenate([res.results[i]["out"] for i in range(N_CORES)], axis=0)
    if trace:
        kernel.last_results = res
    return out.astype(np.float32)
